# revision 9
# baseline (speedup 1.0000x reference)
"""Trainium2 Bass kernel for nn_MultiHeadAttention (x:[2,2048,512], 8 heads, d=64).

Sharding: 8 cores = 2 batches x 4 head-pairs. Each core computes the QKV
projection for its 2 heads, the attention, and a partial (row-split) O
projection. Host sums the 4 partials per batch and adds the output bias.

Per-core on-device layout (all contractions land on SBUF partitions):
  xT  [512, 2048]  = x[b].T                (host pre-transposed)
  QT  [128, 2048]  = wq.T @ xT             (head dims 2x64 on partitions)
  KT  [128, 2048]  = wk.T @ xT
  V   [2048, 128]  = xT.T @ wv             (natural; k-seq on partitions)
  PT  [2048, q]    = exp(scale * KT_h.T @ QT_h)   (scores^T, per head)
  av  [128, q]     = [V_h0 | V_h1].T @ PT_h       (column-packed matmul pairs)
  sums[q]          = ones.T @ PT_h                (M=1 matmuls, col-packed)
  Z   [128, 2048]  = av / sums                    (attn out, heads-transposed)
  out [2048, 512]  = Z.T @ wo                     (partial; host adds across cores)
"""

import sys

import numpy as np

for _p in ("/opt/trn_rl_repo",):
    if _p not in sys.path:
        sys.path.insert(0, _p)

import concourse.bass as bass  # noqa: E402
import concourse.tile as tile  # noqa: E402
from concourse import bacc, mybir  # noqa: E402
from concourse.bass_utils import run_bass_kernel_spmd  # noqa: E402

EMBED = 512
NH = 8
HD = 64
S = 2048
B = 2
SCALE = HD ** -0.5
F32 = mybir.dt.float32

# matmul input dtype: float32r = single-pass (fast, reduced precision mul),
# float32 = 4-pass exact.
MM_DT = mybir.dt.float32r
# f32r matmuls cannot target a dst partition != 0 (no column tiling), so the
# column-packed AV/sums matmuls use bf16 inputs instead
AV_DT = mybir.dt.bfloat16

N_KT = EMBED // 128   # 4 contraction k-tiles for the projections
N_QT = S // 512       # 4 q column tiles
N_ST = S // 128       # 16 seq tiles of 128


def build_nc():
    nc = bacc.Bacc("TRN2", target_bir_lowering=False, debug=False)

    xT_d = nc.dram_tensor("xT", [EMBED, S], MM_DT, kind="ExternalInput").ap()
    wq_d = nc.dram_tensor("wq", [EMBED, 128], MM_DT, kind="ExternalInput").ap()
    wk_d = nc.dram_tensor("wk", [EMBED, 128], MM_DT, kind="ExternalInput").ap()
    wv_d = nc.dram_tensor("wv", [EMBED, 128], MM_DT, kind="ExternalInput").ap()
    wo_d = nc.dram_tensor("wo", [128, EMBED], MM_DT, kind="ExternalInput").ap()
    out_d = nc.dram_tensor("out", [S, EMBED], F32, kind="ExternalOutput").ap()

    with tile.TileContext(nc) as tc:
        with (
            tc.tile_pool(name="persist", bufs=1) as persist,
            tc.tile_pool(name="pt_pool", bufs=4) as pt_pool,
            tc.tile_pool(name="norm", bufs=2) as norm_pool,
            tc.tile_pool(name="ostage", bufs=3) as ostage,
            tc.tile_pool(name="ps", bufs=2, space="PSUM") as ps_pool,
            tc.tile_pool(name="ps_av", bufs=1, space="PSUM") as ps_av_pool,
            tc.tile_pool(name="ps_sum", bufs=1, space="PSUM") as ps_sum_pool,
        ):
            # ---- load phase ----
            xT_sb = persist.tile([128, N_KT, S], MM_DT)  # [part, ktile, seq]
            for k in range(N_KT):
                nc.sync.dma_start(out=xT_sb[:, k, :], in_=xT_d[k * 128:(k + 1) * 128, :])
            wq_sb = persist.tile([128, N_KT, 128], MM_DT)
            wk_sb = persist.tile([128, N_KT, 128], MM_DT)
            wv_sb = persist.tile([128, N_KT, 128], MM_DT)
            for w_sb, w_d in ((wq_sb, wq_d), (wk_sb, wk_d), (wv_sb, wv_d)):
                for k in range(N_KT):
                    nc.sync.dma_start(out=w_sb[:, k, :], in_=w_d[k * 128:(k + 1) * 128, :])
            wo_sb = persist.tile([128, EMBED], MM_DT)
            nc.sync.dma_start(out=wo_sb, in_=wo_d)
            # ones as matmul lhsT with M=64 -> the denominator sums come out of
            # the PE already replicated across 64 partitions (free broadcast)
            ones_sb = persist.tile([128, HD], AV_DT)
            nc.vector.memset(ones_sb, 1.0)

            # ---- qkv projections ----
            KT_sb = persist.tile([128, S], MM_DT)
            QT_sb = persist.tile([128, S], MM_DT)
            for w_sb, t_sb in ((wk_sb, KT_sb), (wq_sb, QT_sb)):
                for qt in range(N_QT):
                    qs = bass.ts(qt, 512)
                    ps = ps_pool.tile([128, 2, 512], F32, tag="ps")
                    for k in range(N_KT):
                        nc.tensor.matmul(
                            ps[:, 0, :],
                            w_sb[:, k, :],
                            xT_sb[:, k, qs],
                            start=(k == 0),
                            stop=(k == N_KT - 1),
                        )
                    nc.vector.tensor_copy(t_sb[:, qs], ps[:, 0, :])
            V_sb = persist.tile([128, N_ST, 128], AV_DT)  # [k-seq within tile, seq-tile, feat]
            for st in range(N_ST):
                ps = ps_pool.tile([128, 2, 512], F32, tag="ps")
                for k in range(N_KT):
                    nc.tensor.matmul(
                        ps[:, 0, 0:128],
                        xT_sb[:, k, bass.ts(st, 128)],
                        wv_sb[:, k, :],
                        start=(k == 0),
                        stop=(k == N_KT - 1),
                    )
                nc.vector.tensor_copy(V_sb[:, st, :], ps[:, 0, 0:128])

            # ---- attention + normalize ----
            Z_sb = persist.tile([128, S], MM_DT)  # normalized attn out^T, both heads
            for qt in range(N_QT):
                qs = bass.ts(qt, 512)
                # one accumulator bank per head (a PSUM bank supports only one
                # accumulation group); h1 slices at partition 64 so the packed
                # matmuls still run on disjoint column groups concurrently
                av0 = ps_av_pool.tile([128, 512], F32, tag="av0")
                av1 = ps_av_pool.tile([128, 512], F32, tag="av1")
                sums0 = ps_sum_pool.tile([128, 512], F32, tag="sums0")
                sums1 = ps_sum_pool.tile([128, 512], F32, tag="sums1")
                for c in range(N_ST // 2):  # chunks of 2 k-subtiles
                    s0 = ps_pool.tile([128, 2, 512], F32, tag="ps")
                    s1 = ps_pool.tile([128, 2, 512], F32, tag="ps")
                    for j in range(2):
                        ks = 2 * c + j
                        kk = bass.ts(ks, 128)
                        # head-paired score matmuls (row groups 0:64 / 64:128)
                        nc.tensor.matmul(
                            s0[:, j, :], KT_sb[0:64, kk], QT_sb[0:64, qs],
                            start=True, stop=True,
                        )
                        nc.tensor.matmul(
                            s1[:, j, :], KT_sb[64:128, kk], QT_sb[64:128, qs],
                            start=True, stop=True,
                        )
                    pt0 = pt_pool.tile([128, 2, 512], AV_DT, tag="pt")
                    pt1 = pt_pool.tile([128, 2, 512], AV_DT, tag="pt")
                    nc.scalar.activation(
                        out=pt0, in_=s0, func=mybir.ActivationFunctionType.Exp,
                        scale=SCALE,
                    )
                    nc.scalar.activation(
                        out=pt1, in_=s1, func=mybir.ActivationFunctionType.Exp,
                        scale=SCALE,
                    )
                    for j in range(2):
                        ks = 2 * c + j
                        first = ks == 0
                        last = ks == N_ST - 1
                        # column-packed AV pairs
                        nc.tensor.matmul(
                            av0[0:64, :], V_sb[:, ks, 0:64], pt0[:, j, :],
                            start=first, stop=last,
                        )
                        nc.tensor.matmul(
                            av1[64:128, :], V_sb[:, ks, 64:128], pt1[:, j, :],
                            start=first, stop=last,
                        )
                        # column-packed softmax-denominator sums, replicated
                        # across 64 partitions by the M=64 all-ones lhsT
                        nc.tensor.matmul(
                            sums0[0:64, :], ones_sb, pt0[:, j, :],
                            start=first, stop=last,
                        )
                        nc.tensor.matmul(
                            sums1[64:128, :], ones_sb, pt1[:, j, :],
                            start=first, stop=last,
                        )
                rb_sb = norm_pool.tile([128, 512], F32, tag="rb")
                nc.vector.reciprocal(out=rb_sb[0:64, :], in_=sums0[0:64, :])
                nc.vector.reciprocal(out=rb_sb[64:128, :], in_=sums1[64:128, :])
                nc.vector.tensor_mul(Z_sb[0:64, qs], av0[0:64, :], rb_sb[0:64, :])
                nc.vector.tensor_mul(Z_sb[64:128, qs], av1[64:128, :], rb_sb[64:128, :])

            # ---- output projection (partial; host sums across head-pairs) ----
            for m in range(N_ST):
                ps = ps_pool.tile([128, 2, 512], F32, tag="ps")
                nc.tensor.matmul(
                    ps[:, 0, :], Z_sb[:, bass.ts(m, 128)], wo_sb,
                    start=True, stop=True,
                )
                ot = ostage.tile([128, 512], F32, tag="ot")
                nc.vector.tensor_copy(ot, ps[:, 0, :])
                nc.sync.dma_start(out=out_d[bass.ts(m, 128), :], in_=ot)

    nc.compile()
    return nc


_NC = None


def _get_nc():
    global _NC
    if _NC is None:
        _NC = build_nc()
    return _NC


def make_in_maps(x, w_qkv, w_o):
    x = np.ascontiguousarray(np.asarray(x, dtype=np.float32))
    w_qkv = np.asarray(w_qkv, dtype=np.float32)
    w_o = np.asarray(w_o, dtype=np.float32)
    in_maps = []
    xTs = [np.ascontiguousarray(x[b].T) for b in range(B)]
    for c in range(8):
        b, g = c // 4, c % 4
        cols = slice(2 * g * HD, (2 * g + 2) * HD)
        in_maps.append({
            "xT": xTs[b],
            "wq": np.ascontiguousarray(w_qkv[:, :EMBED][:, cols]),
            "wk": np.ascontiguousarray(w_qkv[:, EMBED:2 * EMBED][:, cols]),
            "wv": np.ascontiguousarray(w_qkv[:, 2 * EMBED:][:, cols]),
            "wo": np.ascontiguousarray(w_o[cols, :]),
        })
    return in_maps


def combine(results, b_o):
    partials = np.stack([r["out"] for r in results])  # [8, S, EMBED]
    out = partials.reshape(B, 4, S, EMBED).sum(axis=1)
    return (out + np.asarray(b_o, dtype=np.float32)).astype(np.float32)


def kernel(x, w_qkv, w_o, b_o):
    nc = _get_nc()
    res = run_bass_kernel_spmd(nc, make_in_maps(x, w_qkv, w_o), core_ids=list(range(8)))
    return combine(res.results, b_o)


if __name__ == "__main__":
    import jax
    sys.path.insert(0, "/root/problem")
    import reference

    inputs = reference.setup_inputs()
    expected = np.asarray(reference.reference(**inputs))
    actual = kernel(**{k: np.asarray(v) for k, v in inputs.items()})
    err = np.abs(actual - expected).max()
    print("max abs err:", err, " rel:", err / np.abs(expected).max())


# revision 12
# speedup vs baseline: 1.3611x; 1.3611x over previous
"""Trainium2 Bass kernel for nn_MultiHeadAttention (x:[2,2048,512], 8 heads, d=64).

Sharding: 8 cores = 2 batches x 4 head-pairs. Each core computes the QKV
projection for its 2 heads, the attention, and a partial (row-split) O
projection. Host sums the 4 partials per batch and adds the output bias.

Per-core on-device layout (all contractions land on SBUF partitions):
  xT  [512, 2048]  = x[b].T                (host pre-transposed)
  QT  [128, 2048]  = wq.T @ xT             (head dims 2x64 on partitions)
  KT  [128, 2048]  = wk.T @ xT
  V   [2048, 128]  = xT.T @ wv             (natural; k-seq on partitions)
  PT  [2048, q]    = exp(scale * KT_h.T @ QT_h)   (scores^T, per head, bf16)
  avs [128, q]     = [V_h0|V_h1].T @ [PT_h0|PT_h1]  (one PSUM bank, both heads)
  sums[128, q]     = ones64.T @ PT_h  (denominators, PE-replicated per head half)
  Z   [128, 2048]  = avs * recip(sums)              (attn out, heads-transposed)
  out [2048, 512]  = Z.T @ wo                       (partial; host reduces)

PSUM accumulation-group trick: a bank supports one start/stop group, so each
shared bank is opened by an N=1 zero-weight "clear" matmul (start=True) and
closed by another (stop=True); all real matmuls accumulate with start=False
in any schedule order (per-element has_written handles first-write).
"""

import sys

import numpy as np

for _p in ("/opt/trn_rl_repo",):
    if _p not in sys.path:
        sys.path.insert(0, _p)

import concourse.bass as bass  # noqa: E402
import concourse.tile as tile  # noqa: E402
from concourse import bacc, mybir  # noqa: E402
from concourse.bass_utils import run_bass_kernel_spmd  # noqa: E402

EMBED = 512
NH = 8
HD = 64
S = 2048
B = 2
SCALE = HD ** -0.5
F32 = mybir.dt.float32

# float32r: single-pass matmul (tf32-like); cannot target dst partition != 0,
# so the AV/sums matmuls (which write to partition 64) use bf16 inputs.
MM_DT = mybir.dt.float32r
AV_DT = mybir.dt.bfloat16

N_KT = EMBED // 128   # 4 contraction k-tiles for the projections
N_QT = S // 512       # 4 q column tiles
N_ST = S // 128       # 16 seq tiles of 128


def build_nc():
    nc = bacc.Bacc("TRN2", target_bir_lowering=False, debug=False)

    xT_d = nc.dram_tensor("xT", [EMBED, S], MM_DT, kind="ExternalInput").ap()
    wq_d = nc.dram_tensor("wq", [EMBED, 128], MM_DT, kind="ExternalInput").ap()
    wk_d = nc.dram_tensor("wk", [EMBED, 128], MM_DT, kind="ExternalInput").ap()
    wv_d = nc.dram_tensor("wv", [EMBED, 128], MM_DT, kind="ExternalInput").ap()
    wo_d = nc.dram_tensor("wo", [128, EMBED], MM_DT, kind="ExternalInput").ap()
    out_d = nc.dram_tensor("out", [S, EMBED], F32, kind="ExternalOutput").ap()

    with tile.TileContext(nc) as tc:
        with (
            tc.tile_pool(name="persist", bufs=1) as persist,
            tc.tile_pool(name="pt_pool", bufs=4) as pt_pool,
            tc.tile_pool(name="norm", bufs=2) as norm_pool,
            tc.tile_pool(name="ostage", bufs=3) as ostage,
            tc.tile_pool(name="ps", bufs=2, space="PSUM") as ps_pool,
            tc.tile_pool(name="ps_avs", bufs=2, space="PSUM") as ps_avs_pool,
            tc.tile_pool(name="ps_sum", bufs=2, space="PSUM") as ps_sum_pool,
        ):
            # ---- load phase ----
            xT_sb = persist.tile([128, N_KT, S], MM_DT)  # [part, ktile, seq]
            for k in range(N_KT):
                nc.sync.dma_start(out=xT_sb[:, k, :], in_=xT_d[k * 128:(k + 1) * 128, :])
            wq_sb = persist.tile([128, N_KT, 128], MM_DT)
            wk_sb = persist.tile([128, N_KT, 128], MM_DT)
            wv_sb = persist.tile([128, N_KT, 128], MM_DT)
            for w_sb, w_d in ((wq_sb, wq_d), (wk_sb, wk_d), (wv_sb, wv_d)):
                for k in range(N_KT):
                    nc.sync.dma_start(out=w_sb[:, k, :], in_=w_d[k * 128:(k + 1) * 128, :])
            wo_sb = persist.tile([128, EMBED], MM_DT)
            nc.sync.dma_start(out=wo_sb, in_=wo_d)
            # all-ones lhsT (M=64): denominator matmuls write sums replicated
            # across the head's 64 partitions (free PE-side broadcast)
            ones_sb = persist.tile([128, HD], AV_DT)
            nc.vector.memset(ones_sb, 1.0)
            # zero weights/data for the full-bank psum group open/close matmuls
            zeros_sb = persist.tile([128, 512], AV_DT)
            nc.vector.memset(zeros_sb, 0.0)

            # ---- qkv projections ----
            KT_sb = persist.tile([128, S], MM_DT)
            QT_sb = persist.tile([128, S], MM_DT)
            for w_sb, t_sb in ((wk_sb, KT_sb), (wq_sb, QT_sb)):
                for qt in range(N_QT):
                    qs = bass.ts(qt, 512)
                    ps = ps_pool.tile([128, 2, 512], F32, tag="ps")
                    for k in range(N_KT):
                        nc.tensor.matmul(
                            ps[:, 0, :],
                            w_sb[:, k, :],
                            xT_sb[:, k, qs],
                            start=(k == 0),
                            stop=(k == N_KT - 1),
                        )
                    nc.vector.tensor_copy(t_sb[:, qs], ps[:, 0, :])
            V_sb = persist.tile([128, N_ST, 128], AV_DT)  # [k-seq in tile, seq-tile, feat]
            for st in range(N_ST):
                ps = ps_pool.tile([128, 2, 512], F32, tag="ps")
                for k in range(N_KT):
                    nc.tensor.matmul(
                        ps[:, 0, 0:128],
                        xT_sb[:, k, bass.ts(st, 128)],
                        wv_sb[:, k, :],
                        start=(k == 0),
                        stop=(k == N_KT - 1),
                    )
                nc.vector.tensor_copy(V_sb[:, st, :], ps[:, 0, 0:128])

            # ---- attention (+ interleaved O-projection of the previous tile) ----
            Z_sb = persist.tile([128, S], MM_DT)  # normalized attn out^T, both heads

            def clear_mm(bank, start, stop):
                # zero-weight full-bank matmul: opens/closes the bank's psum
                # accumulation group without affecting accumulated values
                nc.tensor.matmul(
                    bank[:, :], zeros_sb[:, 0:128], zeros_sb,
                    start=start, stop=stop,
                )

            for qt in range(N_QT):
                qs = bass.ts(qt, 512)
                avs = ps_avs_pool.tile([128, 512], F32, tag="avs")
                sums = ps_sum_pool.tile([128, 512], F32, tag="sums")
                clear_mm(avs, True, False)
                clear_mm(sums, True, False)
                for c in range(N_ST // 2):  # chunks of 2 k-subtiles
                    s0 = ps_pool.tile([128, 2, 512], F32, tag="ps")
                    s1 = ps_pool.tile([128, 2, 512], F32, tag="ps")
                    for j in range(2):
                        ks = 2 * c + j
                        kk = bass.ts(ks, 128)
                        # adjacent head-paired score matmuls: disjoint row
                        # groups (0:64 / 64:128) overlap on the PE array
                        nc.tensor.matmul(
                            s0[:, j, :], KT_sb[0:64, kk], QT_sb[0:64, qs],
                            start=True, stop=True,
                        )
                        nc.tensor.matmul(
                            s1[:, j, :], KT_sb[64:128, kk], QT_sb[64:128, qs],
                            start=True, stop=True,
                        )
                    pt0 = pt_pool.tile([128, 2, 512], AV_DT, tag="pt")
                    pt1 = pt_pool.tile([128, 2, 512], AV_DT, tag="pt")
                    nc.scalar.activation(
                        out=pt0, in_=s0, func=mybir.ActivationFunctionType.Exp,
                        scale=SCALE,
                    )
                    nc.scalar.activation(
                        out=pt1, in_=s1, func=mybir.ActivationFunctionType.Exp,
                        scale=SCALE,
                    )
                    for j in range(2):
                        ks = 2 * c + j
                        nc.tensor.matmul(
                            avs[0:64, :], V_sb[:, ks, 0:64], pt0[:, j, :],
                            start=False, stop=False,
                        )
                        nc.tensor.matmul(
                            avs[64:128, :], V_sb[:, ks, 64:128], pt1[:, j, :],
                            start=False, stop=False,
                        )
                        nc.tensor.matmul(
                            sums[0:64, :], ones_sb, pt0[:, j, :],
                            start=False, stop=False,
                        )
                        nc.tensor.matmul(
                            sums[64:128, :], ones_sb, pt1[:, j, :],
                            start=False, stop=False,
                        )
                clear_mm(avs, False, True)
                clear_mm(sums, False, True)
                rb_sb = norm_pool.tile([128, 512], F32, tag="rb")
                nc.vector.reciprocal_approx_fast(out=rb_sb, in_=sums)
                nc.vector.tensor_mul(Z_sb[:, qs], avs, rb_sb)
                # O-projection for this q range; the matmuls rotate through the
                # avs psum slots and execute during the next tile's k-loop,
                # keeping the PE array warm across the normalize boundary
                for mi in range(4):
                    m = 4 * qt + mi
                    po = ps_avs_pool.tile([128, 512], F32, tag="avs")
                    nc.tensor.matmul(
                        po, Z_sb[:, bass.ts(m, 128)], wo_sb, start=True, stop=True,
                    )
                    ot = ostage.tile([128, 512], F32, tag="ot")
                    nc.vector.tensor_copy(ot, po)
                    nc.sync.dma_start(out=out_d[bass.ts(m, 128), :], in_=ot)

    nc.compile()
    return nc


_NC = None


def _get_nc():
    global _NC
    if _NC is None:
        _NC = build_nc()
    return _NC


def make_in_maps(x, w_qkv, w_o):
    x = np.ascontiguousarray(np.asarray(x, dtype=np.float32))
    w_qkv = np.asarray(w_qkv, dtype=np.float32)
    w_o = np.asarray(w_o, dtype=np.float32)
    in_maps = []
    xTs = [np.ascontiguousarray(x[b].T) for b in range(B)]
    for c in range(8):
        b, g = c // 4, c % 4
        cols = slice(2 * g * HD, (2 * g + 2) * HD)
        in_maps.append({
            "xT": xTs[b],
            "wq": np.ascontiguousarray(w_qkv[:, :EMBED][:, cols]),
            "wk": np.ascontiguousarray(w_qkv[:, EMBED:2 * EMBED][:, cols]),
            "wv": np.ascontiguousarray(w_qkv[:, 2 * EMBED:][:, cols]),
            "wo": np.ascontiguousarray(w_o[cols, :]),
        })
    return in_maps


def combine(results, b_o):
    partials = np.stack([r["out"] for r in results])  # [8, S, EMBED]
    out = partials.reshape(B, 4, S, EMBED).sum(axis=1)
    return (out + np.asarray(b_o, dtype=np.float32)).astype(np.float32)


def kernel(x, w_qkv, w_o, b_o):
    nc = _get_nc()
    res = run_bass_kernel_spmd(nc, make_in_maps(x, w_qkv, w_o), core_ids=list(range(8)))
    return combine(res.results, b_o)
